# revision 32
# baseline (speedup 1.0000x reference)
"""Trainium2 Bass kernel: low-rank (LoRA-style) linear with 2:4 soft-threshold
pruned weights, fp16 matmul / fp32 accumulate.

  wA = soft_threshold24(weight_A) * scale_A          # [IN, R]
  wB = soft_threshold24(weight_B) * scale_B          # [OUT, R]
  x_proj = f16(x) @ f16(wA)            (f32 accum)   # [N, R]
  out    = f16(x_proj) @ f16(wB).T + bias            # [N, OUT]

Sharding: data-parallel over the token dim across 8 cores (2048 tokens/core),
small weights replicated. No collectives.

Per-core pipeline (8 groups of 256 tokens):
  gpsimd cast-DMA x f32(HBM) -> f16(SBUF) -> PE transpose (f16, via identity)
  -> ACT copy PSUM->SBUF f16 -> 32 accumulating f16 matmuls vs wA (256-wide)
  -> DVE cast to f16 + ones row -> f16 matmuls vs wB.T (bias row folded in)
  -> DVE copy PSUM->SBUF f32 -> sync-engine DMA store.

The weight-B PE transposes are emitted after groups 0/1's transpose+mm1 so
the in-order PE queue is never head-blocked on the DVE threshold chain.
"""

import sys

import numpy as np

if "/opt/trn_rl_repo" not in sys.path:
    sys.path.insert(0, "/opt/trn_rl_repo")

B, S, IN_F, OUT_F, RANK = 4, 4096, 4096, 4096, 64
N_CORES = 8
N_TOK = B * S                   # 16384
T_CORE = N_TOK // N_CORES       # 2048 tokens per core
P = 128
TT = 2                          # token tiles per group
GTOK = TT * P                   # 256 tokens per group
N_GRP = T_CORE // GTOK          # 8 groups per core
N_IB = IN_F // P                # 32 input-feature blocks
MM2_N = 512
N_OB = OUT_F // MM2_N           # 8 output column groups

USE_CAST_DMA = True

_CACHE = {}


def _soft_threshold_weights(nc, pool, w_dram, scale, out_f16, sfx):
    """Emit IR computing out_f16 = f16(soft_threshold24(w_dram) * scale).

    The HOST pre-permutes w_dram (see kernel()): row (p*nb + b) holds
    original row (b*128 + p), and the rank dim is stored quarter-major
    (col q*16+g holds rank 4*g+q).  So the device DMA is fully contiguous
    (one 8KB descriptor per partition), and each quarter-of-4 slice is a
    contiguous [P, nb, 16] block.  The whole chain runs in f16 on DVE
    (weights are cast f32->f16 during the DMA); soft-threshold is
    1-Lipschitz in (w, t) so f16 rounding stays ~1e-3 relative.
    """
    import concourse.mybir as mybir

    f16 = mybir.dt.float16
    nb = w_dram.shape[0] // P
    G = RANK // 4
    wf = pool.tile([P, nb, RANK], f16, tag=f"wstage{sfx}", name="wstage")
    nc.gpsimd.dma_start(wf[:], w_dram[:].rearrange("(p b) r -> p b r", p=P))

    amin = mybir.AluOpType.min
    amx = mybir.AluOpType.max
    ve = nc.vector

    g = wf[:].rearrange("p b (q g) -> p b q g", q=4)
    gj = [g[:, :, j : j + 1, :] for j in range(4)]
    ash = [P, nb, 1, G]

    # a = |w| = max(w, -w)
    wneg = pool.tile([P, nb, RANK], f16, tag="wneg", name="wneg")
    ve.tensor_scalar_mul(wneg[:], wf[:], -1.0)
    a = pool.tile([P, nb, RANK], f16, tag="wabs", name="wabs")
    ve.tensor_tensor(a[:], wf[:], wneg[:], op=amx)
    ag = a[:].rearrange("p b (q g) -> p b q g", q=4)
    aj = [ag[:, :, j : j + 1, :] for j in range(4)]

    m1 = pool.tile(ash, f16, tag="m1", name="m1")
    M1 = pool.tile(ash, f16, tag="M1", name="M1")
    m2 = pool.tile(ash, f16, tag="m2", name="m2")
    M2 = pool.tile(ash, f16, tag="M2", name="M2")
    ve.tensor_tensor(m1[:], aj[0], aj[1], op=amin)
    ve.tensor_tensor(M1[:], aj[0], aj[1], op=amx)
    ve.tensor_tensor(m2[:], aj[2], aj[3], op=amin)
    ve.tensor_tensor(M2[:], aj[2], aj[3], op=amx)
    # threshold t = 2nd smallest of 4 = min(max(m1, m2), min(M1, M2))
    t = pool.tile(ash, f16, tag="thr", name="t")
    nt = pool.tile(ash, f16, tag="nthr", name="nt")
    ve.tensor_tensor(m1[:], m1[:], m2[:], op=amx)
    ve.tensor_tensor(M1[:], M1[:], M2[:], op=amin)
    ve.tensor_tensor(t[:], m1[:], M1[:], op=amin)
    ve.tensor_scalar_mul(nt[:], t[:], -1.0)

    # c = clip(w, -t, t) per quarter (contiguous slices)
    c = pool.tile([P, nb, RANK], f16, tag="wclip", name="wclip")
    cg = c[:].rearrange("p b (q g) -> p b q g", q=4)
    for j in range(4):
        ve.tensor_tensor(cg[:, :, j : j + 1, :], gj[j], t[:], op=amin)
    for j in range(4):
        cj = cg[:, :, j : j + 1, :]
        ve.tensor_tensor(cj, cj, nt[:], op=amx)

    # s = (w - c) * scale
    if scale == 1.0:
        ve.tensor_tensor(out_f16[:], wf[:], c[:], op=mybir.AluOpType.subtract)
    else:
        ve.tensor_tensor(a[:], wf[:], c[:], op=mybir.AluOpType.subtract)
        ve.tensor_scalar_mul(out_f16[:], a[:], float(scale))


def _build(scale_a, scale_b):
    import concourse.mybir as mybir
    import concourse.tile as tile
    from concourse import bacc
    from concourse.bass import ts

    f32, f16 = mybir.dt.float32, mybir.dt.float16

    nc = bacc.Bacc("TRN2", target_bir_lowering=False, debug=False,
                   enable_asserts=False)
    x_d = nc.dram_tensor("x", [T_CORE, IN_F], f32, kind="ExternalInput")
    wa_d = nc.dram_tensor("weight_A", [IN_F, RANK], f32, kind="ExternalInput")
    wb_d = nc.dram_tensor("weight_B", [OUT_F, RANK], f32, kind="ExternalInput")
    b_d = nc.dram_tensor("bias16", [1, OUT_F], f16, kind="ExternalInput")
    id_d = nc.dram_tensor("ident", [P, P], f16, kind="ExternalInput")
    o_d = nc.dram_tensor("out", [T_CORE, OUT_F], f32, kind="ExternalOutput")

    with tile.TileContext(nc) as tc:
        with (
            tc.tile_pool(name="const", bufs=1) as constp,
            tc.tile_pool(name="wtmp", bufs=1) as wtmp,
            tc.tile_pool(name="xin", bufs=6) as xinp,
            tc.tile_pool(name="xtp", bufs=3) as xtp,
            tc.tile_pool(name="outp", bufs=3) as outp,
            tc.tile_pool(name="proj", bufs=4) as projp,
            tc.tile_pool(name="pst", bufs=3, space="PSUM") as pst,
            tc.tile_pool(name="ps1", bufs=1, space="PSUM") as ps1p,
            tc.tile_pool(name="ps2", bufs=4, space="PSUM") as ps2p,
        ):
            # weight cast-DMAs first on the gpsimd queue so the DVE chain
            # starts ASAP; identity + f16 bias are host-fed constants on the
            # idle sync queue.
            wa16 = constp.tile([P, N_IB, RANK], f16)
            _soft_threshold_weights(nc, wtmp, wa_d, scale_a, wa16[:], "a")
            wb16 = constp.tile([P, OUT_F // P, RANK], f16)
            _soft_threshold_weights(nc, wtmp, wb_d, scale_b, wb16[:], "b")

            wbt = constp.tile([RANK + 1, OUT_F], f16)  # wB.T (+ bias row)
            nc.sync.dma_start(wbt[RANK : RANK + 1, :], b_d[:])
            ident16 = constp.tile([P, P], f16)
            nc.sync.dma_start(ident16[:], id_d[:])

            def emit_wb_transpose():
                # f16 transposes write f16 PSUM: 8/bank, one ACT copy per bank
                for b8 in range(OUT_F // (8 * P)):
                    pw = pst.tile([P, 8 * P], f16, tag="ptx", name="pw")
                    for bb in range(8):
                        b = 8 * b8 + bb
                        nc.tensor.transpose(pw[:RANK, ts(bb, P)],
                                            wb16[:, b, :], ident16[:])
                    nc.scalar.copy(wbt[0:RANK, ts(b8, 8 * P)], pw[:RANK, :])

            def emit_front(g):
                """x load + transpose + mm1 + xpa for group g."""
                xts = []
                for tt in range(TT):
                    i = g * TT + tt
                    if USE_CAST_DMA:
                        xt16 = xinp.tile([P, IN_F], f16, name="xt16",
                                         tag="xt16")
                        nc.gpsimd.dma_start(xt16[:], x_d[ts(i, P), :])
                    else:
                        xt32 = xinp.tile([P, IN_F], f32, name="xt32",
                                         tag="xt32")
                        nc.sync.dma_start(xt32[:], x_d[ts(i, P), :])
                        xt16 = xinp.tile([P, IN_F], f16, name="xt16",
                                         tag="xt16")
                        eng = nc.scalar if tt == 0 else nc.vector
                        eng.tensor_copy(xt16[:], xt32[:])
                    xts.append(xt16)

                # [t, in] -> [in, t] on PE in f16 (f16 PSUM out; 8 transposes
                # per PSUM bank, one ACT copy per bank).  ACT owns the mm1
                # feed path, DVE owns the mm2 drain path: that keeps both
                # under the ~20us/group HBM cadence.
                xT = xtp.tile([P, N_IB, GTOK], f16)
                for q in range(N_IB // 8):
                    for tt in range(TT):
                        pt = pst.tile([P, 8 * P], f16, tag="ptx", name="pt")
                        for bb in range(8):
                            b = 8 * q + bb
                            nc.tensor.transpose(pt[:, ts(bb, P)],
                                                xts[tt][:, ts(b, P)],
                                                ident16[:])
                        dst = xT[:, 8 * q : 8 * q + 8, ts(tt, P)]
                        nc.scalar.copy(dst, pt[:].rearrange("p (a b) -> p a b", a=8))

                # mm1: x_projT[r, t] = sum_i wa[i, r] * xT[i, t], 256-wide
                ps1 = ps1p.tile([RANK, GTOK], f32)
                for b in range(N_IB):
                    nc.tensor.matmul(ps1[:], wa16[:, b, :], xT[:, b, :],
                                     start=(b == 0), stop=(b == N_IB - 1))

                xpa = projp.tile([RANK + 1, GTOK], f16)
                nc.vector.tensor_copy(out=xpa[0:RANK, :], in_=ps1[:])
                nc.vector.memset(xpa[RANK : RANK + 1, :], 1.0)
                return xpa

            def emit_back(g, xpa, split_store=False):
                """mm2 + PSUM copy + store for group g.  split_store issues
                half-tile DMAs (used on the last group to shorten the
                serial copy->store tail)."""
                for tt in range(TT):
                    i = g * TT + tt
                    ob = outp.tile([P, OUT_F], f32, name="ob", tag="ob")
                    for j in range(N_OB):
                        ps2 = ps2p.tile([P, MM2_N], f32, tag="ps2", name="ps2")
                        nc.tensor.matmul(ps2[:], xpa[:, ts(tt, P)],
                                         wbt[:, ts(j, MM2_N)],
                                         start=True, stop=True)
                        nc.vector.tensor_copy(out=ob[:, ts(j, MM2_N)],
                                              in_=ps2[:])
                        if split_store and j == N_OB // 2 - 1:
                            nc.sync.dma_start(o_d[ts(i, P), : OUT_F // 2],
                                              ob[:, : OUT_F // 2])
                    if split_store:
                        nc.sync.dma_start(o_d[ts(i, P), OUT_F // 2 :],
                                          ob[:, OUT_F // 2 :])
                    else:
                        nc.sync.dma_start(o_d[ts(i, P), :], ob[:])

            # Skewed pipeline: front(g+1) is emitted before back(g), so the
            # in-order PE queue runs the last groups' transposes+mm1 during
            # the HBM-bound middle phase and the end-drain is only mm2+store.
            # Group 0 front-runs while the DVE threshold chain computes wb16.
            xpa_prev = emit_front(0)
            emit_wb_transpose()
            for g in range(1, N_GRP):
                xpa_g = emit_front(g)
                emit_back(g - 1, xpa_prev)
                xpa_prev = xpa_g
            emit_back(N_GRP - 1, xpa_prev, split_store=True)

    nc.compile()
    return nc


def get_nc(scale_a, scale_b):
    key = (float(scale_a), float(scale_b))
    if key not in _CACHE:
        _CACHE[key] = _build(*key)
    return _CACHE[key]


def permute_weight(w):
    """Host-side layout marshaling for the device weight DMA.

    Device expects row (p*nb + b) = original row (b*128 + p) so each SBUF
    partition reads one contiguous 8KB chunk, and the rank dim stored
    quarter-major (col q*16+g = rank 4*g+q) so the 2:4 groups-of-4 become
    contiguous quarter slices.  Both are pure permutations; mm1/mm2 contract
    over the permuted rank consistently on both sides.
    """
    n, r = w.shape
    nb = n // P
    wp = w.reshape(nb, P, r // 4, 4)          # [b, p, g, q]
    wp = wp.transpose(1, 0, 3, 2)             # [p, b, q, g]
    return np.ascontiguousarray(wp).reshape(n, r)


def kernel(x, weight_A, weight_B, bias, scale_A, scale_B):
    from concourse.bass_utils import run_bass_kernel_spmd

    x = np.ascontiguousarray(np.asarray(x, dtype=np.float32))
    wa = permute_weight(np.asarray(weight_A, dtype=np.float32))
    wb = permute_weight(np.asarray(weight_B, dtype=np.float32))
    bi = np.ascontiguousarray(
        np.asarray(bias, dtype=np.float32).astype(np.float16)).reshape(1, OUT_F)
    ident = np.eye(P, dtype=np.float16)
    sa = float(np.asarray(scale_A))
    sb = float(np.asarray(scale_B))

    nc = get_nc(sa, sb)

    xf = x.reshape(N_TOK, IN_F)
    in_maps = [
        {
            "x": xf[c * T_CORE : (c + 1) * T_CORE],
            "weight_A": wa,
            "weight_B": wb,
            "bias16": bi,
            "ident": ident,
        }
        for c in range(N_CORES)
    ]
    res = run_bass_kernel_spmd(nc, in_maps, core_ids=list(range(N_CORES)))
    out = np.concatenate([r["out"] for r in res.results], axis=0)
    return out.reshape(B, S, OUT_F)


# revision 36
# speedup vs baseline: 1.0694x; 1.0694x over previous
"""Trainium2 Bass kernel: low-rank (LoRA-style) linear with 2:4 soft-threshold
pruned weights, fp16 matmul / fp32 accumulate.

  wA = soft_threshold24(weight_A) * scale_A          # [IN, R]
  wB = soft_threshold24(weight_B) * scale_B          # [OUT, R]
  x_proj = f16(x) @ f16(wA)            (f32 accum)   # [N, R]
  out    = f16(x_proj) @ f16(wB).T + bias            # [N, OUT]

Sharding: data-parallel over the token dim across 8 cores (2048 tokens/core),
small weights replicated. No collectives.

Per-core pipeline (8 groups of 256 tokens):
  gpsimd cast-DMA x f32(HBM) -> f16(SBUF) -> PE transpose (f16, via identity)
  -> ACT copy PSUM->SBUF f16 -> 32 accumulating f16 matmuls vs wA (256-wide)
  -> DVE cast to f16 + ones row -> f16 matmuls vs wB.T (bias row folded in)
  -> DVE copy PSUM->SBUF f32 -> sync-engine DMA store.

The weight-B PE transposes are emitted after groups 0/1's transpose+mm1 so
the in-order PE queue is never head-blocked on the DVE threshold chain.
"""

import sys

import numpy as np

if "/opt/trn_rl_repo" not in sys.path:
    sys.path.insert(0, "/opt/trn_rl_repo")

B, S, IN_F, OUT_F, RANK = 4, 4096, 4096, 4096, 64
N_CORES = 8
N_TOK = B * S                   # 16384
T_CORE = N_TOK // N_CORES       # 2048 tokens per core
P = 128
TT = 2                          # token tiles per group
GTOK = TT * P                   # 256 tokens per group
N_GRP = T_CORE // GTOK          # 8 groups per core
N_IB = IN_F // P                # 32 input-feature blocks
MM2_N = 512
N_OB = OUT_F // MM2_N           # 8 output column groups

USE_CAST_DMA = True

_CACHE = {}


def _soft_threshold_weights(nc, pool, w_dram, scale, out_f16, sfx):
    """Emit IR computing out_f16 = f16(soft_threshold24(w_dram) * scale).

    The HOST pre-permutes w_dram (see kernel()): row (p*nb + b) holds
    original row (b*128 + p), and the rank dim is stored quarter-major
    (col q*16+g holds rank 4*g+q).  So the device DMA is fully contiguous
    (one 8KB descriptor per partition), and each quarter-of-4 slice is a
    contiguous [P, nb, 16] block.  The whole chain runs in f16 on DVE
    (weights are cast f32->f16 during the DMA); soft-threshold is
    1-Lipschitz in (w, t) so f16 rounding stays ~1e-3 relative.
    """
    import concourse.mybir as mybir

    f16 = mybir.dt.float16
    nb = w_dram.shape[0] // P
    G = RANK // 4
    wf = pool.tile([P, nb, RANK], f16, tag=f"wstage{sfx}", name="wstage")
    nc.gpsimd.dma_start(wf[:], w_dram[:].rearrange("(p b) r -> p b r", p=P))

    amin = mybir.AluOpType.min
    amx = mybir.AluOpType.max
    ve = nc.vector

    g = wf[:].rearrange("p b (q g) -> p b q g", q=4)
    gj = [g[:, :, j : j + 1, :] for j in range(4)]
    ash = [P, nb, 1, G]

    # a = |w| = max(w, -w)
    wneg = pool.tile([P, nb, RANK], f16, tag="wneg", name="wneg")
    ve.tensor_scalar_mul(wneg[:], wf[:], -1.0)
    a = pool.tile([P, nb, RANK], f16, tag="wabs", name="wabs")
    ve.tensor_tensor(a[:], wf[:], wneg[:], op=amx)
    ag = a[:].rearrange("p b (q g) -> p b q g", q=4)
    aj = [ag[:, :, j : j + 1, :] for j in range(4)]

    m1 = pool.tile(ash, f16, tag="m1", name="m1")
    M1 = pool.tile(ash, f16, tag="M1", name="M1")
    m2 = pool.tile(ash, f16, tag="m2", name="m2")
    M2 = pool.tile(ash, f16, tag="M2", name="M2")
    ve.tensor_tensor(m1[:], aj[0], aj[1], op=amin)
    ve.tensor_tensor(M1[:], aj[0], aj[1], op=amx)
    ve.tensor_tensor(m2[:], aj[2], aj[3], op=amin)
    ve.tensor_tensor(M2[:], aj[2], aj[3], op=amx)
    # threshold t = 2nd smallest of 4 = min(max(m1, m2), min(M1, M2))
    t = pool.tile(ash, f16, tag="thr", name="t")
    nt = pool.tile(ash, f16, tag="nthr", name="nt")
    ve.tensor_tensor(m1[:], m1[:], m2[:], op=amx)
    ve.tensor_tensor(M1[:], M1[:], M2[:], op=amin)
    ve.tensor_tensor(t[:], m1[:], M1[:], op=amin)
    ve.tensor_scalar_mul(nt[:], t[:], -1.0)

    # c = clip(w, -t, t) per quarter (contiguous slices)
    c = pool.tile([P, nb, RANK], f16, tag="wclip", name="wclip")
    cg = c[:].rearrange("p b (q g) -> p b q g", q=4)
    for j in range(4):
        ve.tensor_tensor(cg[:, :, j : j + 1, :], gj[j], t[:], op=amin)
    for j in range(4):
        cj = cg[:, :, j : j + 1, :]
        ve.tensor_tensor(cj, cj, nt[:], op=amx)

    # s = (w - c) * scale
    if scale == 1.0:
        ve.tensor_tensor(out_f16[:], wf[:], c[:], op=mybir.AluOpType.subtract)
    else:
        ve.tensor_tensor(a[:], wf[:], c[:], op=mybir.AluOpType.subtract)
        ve.tensor_scalar_mul(out_f16[:], a[:], float(scale))


def _build(scale_a, scale_b):
    import concourse.mybir as mybir
    import concourse.tile as tile
    from concourse import bacc
    from concourse.bass import ts

    f32, f16 = mybir.dt.float32, mybir.dt.float16

    nc = bacc.Bacc("TRN2", target_bir_lowering=False, debug=False,
                   enable_asserts=False)
    x_d = nc.dram_tensor("x", [T_CORE, IN_F], f32, kind="ExternalInput")
    wa_d = nc.dram_tensor("weight_A", [IN_F, RANK], f32, kind="ExternalInput")
    wb_d = nc.dram_tensor("weight_B", [OUT_F, RANK], f32, kind="ExternalInput")
    b_d = nc.dram_tensor("bias16", [1, OUT_F], f16, kind="ExternalInput")
    id_d = nc.dram_tensor("ident", [P, P], f16, kind="ExternalInput")
    o_d = nc.dram_tensor("out", [T_CORE, OUT_F], f32, kind="ExternalOutput")

    with tile.TileContext(nc) as tc:
        with (
            tc.tile_pool(name="const", bufs=1) as constp,
            tc.tile_pool(name="wtmp", bufs=1) as wtmp,
            tc.tile_pool(name="xin", bufs=3) as xinp,
            tc.tile_pool(name="xtp", bufs=3) as xtp,
            tc.tile_pool(name="outp", bufs=3) as outp,
            tc.tile_pool(name="proj", bufs=4) as projp,
            tc.tile_pool(name="pst", bufs=3, space="PSUM") as pst,
            tc.tile_pool(name="ps1", bufs=1, space="PSUM") as ps1p,
            tc.tile_pool(name="ps2", bufs=4, space="PSUM") as ps2p,
        ):
            # weight cast-DMAs first on the gpsimd queue so the DVE chain
            # starts ASAP; identity + f16 bias are host-fed constants on the
            # idle sync queue.
            wa16 = constp.tile([P, N_IB, RANK], f16)
            _soft_threshold_weights(nc, wtmp, wa_d, scale_a, wa16[:], "a")
            wb16 = constp.tile([P, OUT_F // P, RANK], f16)
            _soft_threshold_weights(nc, wtmp, wb_d, scale_b, wb16[:], "b")

            wbt = constp.tile([RANK + 1, OUT_F], f16)  # wB.T (+ bias row)
            nc.sync.dma_start(wbt[RANK : RANK + 1, :], b_d[:])
            ident16 = constp.tile([P, P], f16)
            nc.sync.dma_start(ident16[:], id_d[:])

            def emit_wb_transpose():
                # f16 transposes write f16 PSUM: 8/bank, one ACT copy per bank
                for b8 in range(OUT_F // (8 * P)):
                    pw = pst.tile([P, 8 * P], f16, tag="ptx", name="pw")
                    for bb in range(8):
                        b = 8 * b8 + bb
                        nc.tensor.transpose(pw[:RANK, ts(bb, P)],
                                            wb16[:, b, :], ident16[:])
                    nc.scalar.copy(wbt[0:RANK, ts(b8, 8 * P)], pw[:RANK, :])

            def emit_front(g):
                """x load + transpose + mm1 + xpa for group g."""
                # one fused cast-DMA per group: halves SWDGE issue and
                # drain overhead on the gpsimd queue; still 2x16KB
                # contiguous chunks per partition.
                xg = xinp.tile([P, TT, IN_F], f16, name="xg", tag="xg")
                nc.gpsimd.dma_start(
                    xg[:],
                    x_d[ts(g, GTOK), :].rearrange("(t p) f -> p t f", p=P))

                # [t, in] -> [in, t] on PE in f16 (f16 PSUM out; 8 transposes
                # per PSUM bank, one ACT copy per bank).  ACT owns the mm1
                # feed path, DVE owns the mm2 drain path: that keeps both
                # under the ~20us/group HBM cadence.
                xT = xtp.tile([P, N_IB, GTOK], f16)
                for q in range(N_IB // 8):
                    for tt in range(TT):
                        pt = pst.tile([P, 8 * P], f16, tag="ptx", name="pt")
                        for bb in range(8):
                            b = 8 * q + bb
                            nc.tensor.transpose(pt[:, ts(bb, P)],
                                                xg[:, tt, ts(b, P)],
                                                ident16[:])
                        dst = xT[:, 8 * q : 8 * q + 8, ts(tt, P)]
                        nc.scalar.copy(dst, pt[:].rearrange("p (a b) -> p a b", a=8))

                # mm1: x_projT[r, t] = sum_i wa[i, r] * xT[i, t], 256-wide
                ps1 = ps1p.tile([RANK, GTOK], f32)
                for b in range(N_IB):
                    nc.tensor.matmul(ps1[:], wa16[:, b, :], xT[:, b, :],
                                     start=(b == 0), stop=(b == N_IB - 1))

                xpa = projp.tile([RANK + 1, GTOK], f16)
                nc.vector.tensor_copy(out=xpa[0:RANK, :], in_=ps1[:])
                nc.vector.memset(xpa[RANK : RANK + 1, :], 1.0)
                return xpa

            def emit_back(g, xpa, split_store=False):
                """mm2 + PSUM copy + store for group g.  split_store issues
                half-tile DMAs (used on the last group to shorten the
                serial copy->store tail)."""
                for tt in range(TT):
                    i = g * TT + tt
                    ob = outp.tile([P, OUT_F], f32, name="ob", tag="ob")
                    for j in range(N_OB):
                        ps2 = ps2p.tile([P, MM2_N], f32, tag="ps2", name="ps2")
                        nc.tensor.matmul(ps2[:], xpa[:, ts(tt, P)],
                                         wbt[:, ts(j, MM2_N)],
                                         start=True, stop=True)
                        nc.vector.tensor_copy(out=ob[:, ts(j, MM2_N)],
                                              in_=ps2[:])
                        if split_store and j == N_OB // 2 - 1:
                            nc.sync.dma_start(o_d[ts(i, P), : OUT_F // 2],
                                              ob[:, : OUT_F // 2])
                    if split_store:
                        nc.sync.dma_start(o_d[ts(i, P), OUT_F // 2 :],
                                          ob[:, OUT_F // 2 :])
                    else:
                        nc.sync.dma_start(o_d[ts(i, P), :], ob[:])

            # Skewed pipeline: front(g+1) is emitted before back(g), so the
            # in-order PE queue runs the last groups' transposes+mm1 during
            # the HBM-bound middle phase and the end-drain is only mm2+store.
            # Group 0 front-runs while the DVE threshold chain computes wb16.
            xpa_prev = emit_front(0)
            emit_wb_transpose()
            for g in range(1, N_GRP):
                xpa_g = emit_front(g)
                emit_back(g - 1, xpa_prev)
                xpa_prev = xpa_g
            emit_back(N_GRP - 1, xpa_prev, split_store=True)

    nc.compile()
    return nc


def get_nc(scale_a, scale_b):
    key = (float(scale_a), float(scale_b))
    if key not in _CACHE:
        _CACHE[key] = _build(*key)
    return _CACHE[key]


def permute_weight(w):
    """Host-side layout marshaling for the device weight DMA.

    Device expects row (p*nb + b) = original row (b*128 + p) so each SBUF
    partition reads one contiguous 8KB chunk, and the rank dim stored
    quarter-major (col q*16+g = rank 4*g+q) so the 2:4 groups-of-4 become
    contiguous quarter slices.  Both are pure permutations; mm1/mm2 contract
    over the permuted rank consistently on both sides.
    """
    n, r = w.shape
    nb = n // P
    wp = w.reshape(nb, P, r // 4, 4)          # [b, p, g, q]
    wp = wp.transpose(1, 0, 3, 2)             # [p, b, q, g]
    return np.ascontiguousarray(wp).reshape(n, r)


def kernel(x, weight_A, weight_B, bias, scale_A, scale_B):
    from concourse.bass_utils import run_bass_kernel_spmd

    x = np.ascontiguousarray(np.asarray(x, dtype=np.float32))
    wa = permute_weight(np.asarray(weight_A, dtype=np.float32))
    wb = permute_weight(np.asarray(weight_B, dtype=np.float32))
    bi = np.ascontiguousarray(
        np.asarray(bias, dtype=np.float32).astype(np.float16)).reshape(1, OUT_F)
    ident = np.eye(P, dtype=np.float16)
    sa = float(np.asarray(scale_A))
    sb = float(np.asarray(scale_B))

    nc = get_nc(sa, sb)

    xf = x.reshape(N_TOK, IN_F)
    in_maps = [
        {
            "x": xf[c * T_CORE : (c + 1) * T_CORE],
            "weight_A": wa,
            "weight_B": wb,
            "bias16": bi,
            "ident": ident,
        }
        for c in range(N_CORES)
    ]
    res = run_bass_kernel_spmd(nc, in_maps, core_ids=list(range(N_CORES)))
    out = np.concatenate([r["out"] for r in res.results], axis=0)
    return out.reshape(B, S, OUT_F)
